# revision 3
# baseline (speedup 1.0000x reference)
"""Trainium2 Bass kernel for nn_CognitiveWorkspaceTransformer.

Math (reference semantics):
    X   = S + concat(w_spoke, w_hub_priv, w_hub_shared, tag)   # full 1088 cover
    out = X @ W_read.T          # (B,T,1024)
    k   = latent @ Wk.T         # cache is fully overwritten by latent
    v   = latent @ Wv.T

Sharding: data-parallel over batch B=8, one batch element per NeuronCore.
Weights (transposed on host, pure layout) are broadcast to all cores.
"""

import numpy as np

import concourse.bass as bass
import concourse.bacc as bacc
import concourse.mybir as mybir
import concourse.tile as tile
from concourse.bass_utils import run_bass_kernel_spmd
from concourse.masks import make_identity

B, T, D_STATE, D_MODEL, D_LATENT = 8, 4096, 1088, 1024, 128
N_CORES = 8
P = 128  # partitions / token tile
F32 = mybir.dt.float32
F32R = mybir.dt.float32r

# feature chunks of the contraction dim (1088 = 8*128 + 64)
R_CHUNKS = [(j * 128, min(128, D_STATE - j * 128)) for j in range((D_STATE + 127) // 128)]

_NC_CACHE = {}


def build_nc(mm_dt=F32R, t_tile=128):
    """Build + compile the per-core Bass program (identical on all cores)."""
    nt = T // t_tile          # outer iterations
    ng = t_tile // P          # 128-row groups per iteration

    nc = bacc.Bacc("TRN2", target_bir_lowering=False, debug=False, num_devices=N_CORES)

    s_d = nc.dram_tensor("s", [T, D_STATE], F32, kind="ExternalInput").ap()
    wc_d = nc.dram_tensor("wc", [T, D_STATE], F32, kind="ExternalInput").ap()
    lat_d = nc.dram_tensor("lat", [T, D_LATENT], F32, kind="ExternalInput").ap()
    wrt_d = nc.dram_tensor("wrt", [D_STATE, D_MODEL], mm_dt, kind="ExternalInput").ap()
    wkt_d = nc.dram_tensor("wkt", [D_LATENT, D_MODEL], mm_dt, kind="ExternalInput").ap()
    wvt_d = nc.dram_tensor("wvt", [D_LATENT, D_MODEL], mm_dt, kind="ExternalInput").ap()
    out_d = nc.dram_tensor("out", [T, D_MODEL], F32, kind="ExternalOutput").ap()
    k_d = nc.dram_tensor("k", [T, D_MODEL], F32, kind="ExternalOutput").ap()
    v_d = nc.dram_tensor("v", [T, D_MODEL], F32, kind="ExternalOutput").ap()

    with tile.TileContext(nc) as tc:
        with (
            tc.tile_pool(name="weights", bufs=1) as wpool,
            tc.tile_pool(name="ident", bufs=1) as ipool,
            tc.tile_pool(name="ins", bufs=3) as inpool,
            tc.tile_pool(name="xt", bufs=2) as xtpool,
            tc.tile_pool(name="outs", bufs=3) as outpool,
            tc.tile_pool(name="psum_t", bufs=2, space="PSUM") as pt_pool,
            tc.tile_pool(name="psum_mm", bufs=6, space="PSUM") as mm_pool,
        ):
            ident = ipool.tile([P, P], F32)
            make_identity(nc, ident[:])

            # resident weights
            wr_tiles = []
            for j, (r0, rw) in enumerate(R_CHUNKS):
                wt = wpool.tile([rw, D_MODEL], mm_dt, tag=f"wr{j}")
                nc.sync.dma_start(wt[:], wrt_d[r0 : r0 + rw, :])
                wr_tiles.append(wt)
            wk_t = wpool.tile([D_LATENT, D_MODEL], mm_dt, tag="wk")
            nc.sync.dma_start(wk_t[:], wkt_d[:])
            wv_t = wpool.tile([D_LATENT, D_MODEL], mm_dt, tag="wv")
            nc.sync.dma_start(wv_t[:], wvt_d[:])

            for it in range(nt):
                t0 = it * t_tile
                s_t = inpool.tile([P, ng, D_STATE], F32, tag="s")
                wc_t = inpool.tile([P, ng, D_STATE], F32, tag="wc")
                la_t = inpool.tile([P, ng, D_LATENT], F32, tag="la")
                nc.sync.dma_start(
                    s_t[:], s_d[t0 : t0 + t_tile, :].rearrange("(n p) d -> p n d", p=P)
                )
                nc.sync.dma_start(
                    wc_t[:], wc_d[t0 : t0 + t_tile, :].rearrange("(n p) d -> p n d", p=P)
                )
                nc.sync.dma_start(
                    la_t[:], lat_d[t0 : t0 + t_tile, :].rearrange("(n p) d -> p n d", p=P)
                )

                # X = S + wcat  (one elementwise add over the full feature dim)
                nc.vector.tensor_add(s_t[:], s_t[:], wc_t[:])

                for g in range(ng):
                    # ---- transposes: X^T chunks and latent^T ----
                    xT = xtpool.tile([P, len(R_CHUNKS), P], mm_dt, tag="xT")
                    for j, (r0, rw) in enumerate(R_CHUNKS):
                        tp = pt_pool.tile([P, P], F32, tag="tp")
                        nc.tensor.transpose(
                            tp[0:rw, :], s_t[:, g, r0 : r0 + rw], ident[:]
                        )
                        nc.vector.tensor_copy(xT[0:rw, j, :], tp[0:rw, :])
                    lT = xtpool.tile([P, P], mm_dt, tag="lT")
                    tp = pt_pool.tile([P, P], F32, tag="tp")
                    nc.tensor.transpose(tp[:], la_t[:, g, :], ident[:])
                    nc.vector.tensor_copy(lT[:], tp[:])

                    # ---- main GEMM: out[t, :] = X @ W_read^T ----
                    out_sb = outpool.tile([P, D_MODEL], F32, tag="out")
                    for h in range(2):
                        n0 = h * 512
                        po = mm_pool.tile([P, 512], F32, tag="mm")
                        for j, (r0, rw) in enumerate(R_CHUNKS):
                            nc.tensor.matmul(
                                po[:],
                                xT[0:rw, j, :],
                                wr_tiles[j][0:rw, n0 : n0 + 512],
                                start=(j == 0),
                                stop=(j == len(R_CHUNKS) - 1),
                            )
                        nc.vector.tensor_copy(out_sb[:, n0 : n0 + 512], po[:])

                    # ---- k/v: latent @ Wk^T, latent @ Wv^T ----
                    k_sb = outpool.tile([P, D_MODEL], F32, tag="k")
                    v_sb = outpool.tile([P, D_MODEL], F32, tag="v")
                    for h in range(2):
                        n0 = h * 512
                        pk = mm_pool.tile([P, 512], F32, tag="mm")
                        nc.tensor.matmul(
                            pk[:], lT[:], wk_t[:, n0 : n0 + 512],
                            start=True, stop=True,
                        )
                        nc.scalar.copy(k_sb[:, n0 : n0 + 512], pk[:])
                        pv = mm_pool.tile([P, 512], F32, tag="mm")
                        nc.tensor.matmul(
                            pv[:], lT[:], wv_t[:, n0 : n0 + 512],
                            start=True, stop=True,
                        )
                        nc.scalar.copy(v_sb[:, n0 : n0 + 512], pv[:])

                    row0 = t0 + g * P
                    nc.scalar.dma_start(out_d[row0 : row0 + P, :], out_sb[:])
                    nc.scalar.dma_start(k_d[row0 : row0 + P, :], k_sb[:])
                    nc.scalar.dma_start(v_d[row0 : row0 + P, :], v_sb[:])

    nc.compile()
    return nc


def _get_nc(**kw):
    key = tuple(sorted(kw.items()))
    if key not in _NC_CACHE:
        _NC_CACHE[key] = build_nc(**kw)
    return _NC_CACHE[key]


def kernel(S, w_spoke, w_hub_priv, w_hub_shared, tag, W_read, cache, latent, Wk, Wv,
           **build_kw):
    S = np.ascontiguousarray(np.asarray(S, np.float32))
    latent = np.ascontiguousarray(np.asarray(latent, np.float32))
    wcat = np.concatenate(
        [np.asarray(w_spoke, np.float32), np.asarray(w_hub_priv, np.float32),
         np.asarray(w_hub_shared, np.float32), np.asarray(tag, np.float32)],
        axis=-1,
    )
    wrt = np.ascontiguousarray(np.asarray(W_read, np.float32).T)
    wkt = np.ascontiguousarray(np.asarray(Wk, np.float32).T)
    wvt = np.ascontiguousarray(np.asarray(Wv, np.float32).T)

    nc = _get_nc(**build_kw)
    in_maps = [
        {"s": S[i], "wc": wcat[i], "lat": latent[i], "wrt": wrt, "wkt": wkt, "wvt": wvt}
        for i in range(N_CORES)
    ]
    res = run_bass_kernel_spmd(nc, in_maps, list(range(N_CORES)))
    out = np.stack([res.results[i]["out"] for i in range(N_CORES)])
    k = np.stack([res.results[i]["k"] for i in range(N_CORES)])
    v = np.stack([res.results[i]["v"] for i in range(N_CORES)])
    return (out, k, v)


# revision 8
# speedup vs baseline: 1.2166x; 1.2166x over previous
"""Trainium2 Bass kernel for nn_CognitiveWorkspaceTransformer.

Math (reference semantics):
    X   = S + concat(w_spoke, w_hub_priv, w_hub_shared, tag)   # full 1088 cover
    out = X @ W_read.T          # (B,T,1024)
    k   = latent @ Wk.T         # cache is fully overwritten by latent
    v   = latent @ Wv.T

Sharding: data-parallel over batch B=8, one batch element per NeuronCore.
All tensors are laid out feature-major on the host (pure layout prep, no
arithmetic) so the contraction dim lands on SBUF partitions directly and
the PE needs no on-chip transposes.
"""

import numpy as np

import concourse.bacc as bacc
import concourse.mybir as mybir
import concourse.tile as tile
from concourse.bass_utils import run_bass_kernel_spmd

B, T, D_STATE, D_MODEL, D_LATENT = 8, 4096, 1088, 1024, 128
N_CORES = 8
P = 128
F32 = mybir.dt.float32
F32R = mybir.dt.float32r

# feature chunks of the contraction dim (1088 = 8*128 + 64)
R_CHUNKS = [(j * 128, min(128, D_STATE - j * 128)) for j in range((D_STATE + 127) // 128)]
NJ = len(R_CHUNKS)

_NC_CACHE = {}


def build_nc(mm_dt=F32R, t_chunk=512, in_bufs=2, out_bufs=3, mm_bufs=8):
    """Build + compile the per-core Bass program (identical on all cores)."""
    nt = T // t_chunk         # outer iterations
    ng = t_chunk // P         # 128-token groups per iteration

    nc = bacc.Bacc("TRN2", target_bir_lowering=False, debug=False, num_devices=N_CORES)

    # feature-major inputs: sT/wcT [1088, T], latT [128, T]
    st_d = nc.dram_tensor("st", [D_STATE, T], mm_dt, kind="ExternalInput").ap()
    wct_d = nc.dram_tensor("wct", [D_STATE, T], mm_dt, kind="ExternalInput").ap()
    latt_d = nc.dram_tensor("latt", [D_LATENT, T], mm_dt, kind="ExternalInput").ap()
    wrt_d = nc.dram_tensor("wrt", [D_STATE, D_MODEL], mm_dt, kind="ExternalInput").ap()
    wkt_d = nc.dram_tensor("wkt", [D_LATENT, D_MODEL], mm_dt, kind="ExternalInput").ap()
    wvt_d = nc.dram_tensor("wvt", [D_LATENT, D_MODEL], mm_dt, kind="ExternalInput").ap()
    out_d = nc.dram_tensor("out", [T, D_MODEL], F32, kind="ExternalOutput").ap()
    k_d = nc.dram_tensor("k", [T, D_MODEL], F32, kind="ExternalOutput").ap()
    v_d = nc.dram_tensor("v", [T, D_MODEL], F32, kind="ExternalOutput").ap()

    with tile.TileContext(nc) as tc:
        with (
            tc.tile_pool(name="weights", bufs=1) as wpool,
            tc.tile_pool(name="ins", bufs=in_bufs) as inpool,
            tc.tile_pool(name="outs", bufs=out_bufs) as outpool,
            tc.tile_pool(name="psum_mm", bufs=mm_bufs, space="PSUM") as mm_pool,
        ):
            # resident weights
            wr_tiles = []
            for j, (r0, rw) in enumerate(R_CHUNKS):
                wt = wpool.tile([rw, D_MODEL], mm_dt, tag=f"wr{j}")
                nc.sync.dma_start(wt[:], wrt_d[r0 : r0 + rw, :])
                wr_tiles.append(wt)
            wk_t = wpool.tile([D_LATENT, D_MODEL], mm_dt, tag="wk")
            nc.sync.dma_start(wk_t[:], wkt_d[:])
            wv_t = wpool.tile([D_LATENT, D_MODEL], mm_dt, tag="wv")
            nc.sync.dma_start(wv_t[:], wvt_d[:])

            for it in range(nt):
                t0 = it * t_chunk
                # X^T tile: [128 (r within chunk), 9 chunks, t_chunk]
                xt = inpool.tile([P, NJ, t_chunk], mm_dt, tag="x")
                wc = inpool.tile([P, NJ, t_chunk], mm_dt, tag="wc")
                lt = inpool.tile([P, t_chunk], mm_dt, tag="lt")
                nc.sync.dma_start(
                    xt[:, 0:8, :],
                    st_d[0:1024, t0 : t0 + t_chunk].rearrange("(j p) t -> p j t", p=P),
                )
                nc.sync.dma_start(xt[0:64, 8, :], st_d[1024:1088, t0 : t0 + t_chunk])
                nc.sync.dma_start(
                    wc[:, 0:8, :],
                    wct_d[0:1024, t0 : t0 + t_chunk].rearrange("(j p) t -> p j t", p=P),
                )
                nc.sync.dma_start(wc[0:64, 8, :], wct_d[1024:1088, t0 : t0 + t_chunk])
                nc.sync.dma_start(lt[:], latt_d[:, t0 : t0 + t_chunk])

                # X = S + wcat (feature-major elementwise; operands of the
                # fp32r matmul must be produced as fp32r -> cast via output)
                xr = xt[:]
                nc.vector.tensor_add(xr, xt[:], wc[:])
                ltr = lt[:]

                for g in range(ng):
                    ts0 = g * P
                    out_sb = outpool.tile([P, D_MODEL], F32, tag="out")
                    for h in range(2):
                        n0 = h * 512
                        po = mm_pool.tile([P, 512], F32, tag="mm")
                        for j, (r0, rw) in enumerate(R_CHUNKS):
                            nc.tensor.matmul(
                                po[:],
                                xr[0:rw, j, ts0 : ts0 + P],
                                wr_tiles[j][0:rw, n0 : n0 + 512],
                                start=(j == 0),
                                stop=(j == NJ - 1),
                            )
                        nc.vector.tensor_copy(out_sb[:, n0 : n0 + 512], po[:])

                    k_sb = outpool.tile([P, D_MODEL], F32, tag="k")
                    v_sb = outpool.tile([P, D_MODEL], F32, tag="v")
                    for h in range(2):
                        n0 = h * 512
                        pk = mm_pool.tile([P, 512], F32, tag="mm")
                        nc.tensor.matmul(
                            pk[:], ltr[:, ts0 : ts0 + P], wk_t[:, n0 : n0 + 512],
                            start=True, stop=True,
                        )
                        nc.scalar.copy(k_sb[:, n0 : n0 + 512], pk[:])
                        pv = mm_pool.tile([P, 512], F32, tag="mm")
                        nc.tensor.matmul(
                            pv[:], ltr[:, ts0 : ts0 + P], wv_t[:, n0 : n0 + 512],
                            start=True, stop=True,
                        )
                        nc.scalar.copy(v_sb[:, n0 : n0 + 512], pv[:])

                    row0 = t0 + ts0
                    nc.scalar.dma_start(out_d[row0 : row0 + P, :], out_sb[:])
                    nc.scalar.dma_start(k_d[row0 : row0 + P, :], k_sb[:])
                    nc.scalar.dma_start(v_d[row0 : row0 + P, :], v_sb[:])

    nc.compile()
    return nc


def _get_nc(**kw):
    key = tuple(sorted(kw.items()))
    if key not in _NC_CACHE:
        _NC_CACHE[key] = build_nc(**kw)
    return _NC_CACHE[key]


def make_in_maps(S, w_spoke, w_hub_priv, w_hub_shared, tag, W_read, cache, latent,
                 Wk, Wv):
    # host-side layout prep only (shard over batch, feature-major transposes)
    wcat = np.concatenate(
        [np.asarray(w_spoke, np.float32), np.asarray(w_hub_priv, np.float32),
         np.asarray(w_hub_shared, np.float32), np.asarray(tag, np.float32)],
        axis=-1,
    )
    sT = np.ascontiguousarray(np.asarray(S, np.float32).transpose(0, 2, 1))
    wcT = np.ascontiguousarray(wcat.transpose(0, 2, 1))
    latT = np.ascontiguousarray(np.asarray(latent, np.float32).transpose(0, 2, 1))
    wrt = np.ascontiguousarray(np.asarray(W_read, np.float32).T)
    wkt = np.ascontiguousarray(np.asarray(Wk, np.float32).T)
    wvt = np.ascontiguousarray(np.asarray(Wv, np.float32).T)
    return [
        {"st": sT[i], "wct": wcT[i], "latt": latT[i],
         "wrt": wrt, "wkt": wkt, "wvt": wvt}
        for i in range(N_CORES)
    ]


def kernel(S, w_spoke, w_hub_priv, w_hub_shared, tag, W_read, cache, latent, Wk, Wv,
           **build_kw):
    in_maps = make_in_maps(S, w_spoke, w_hub_priv, w_hub_shared, tag, W_read, cache,
                           latent, Wk, Wv)
    nc = _get_nc(**build_kw)
    res = run_bass_kernel_spmd(nc, in_maps, list(range(N_CORES)))
    out = np.stack([res.results[i]["out"] for i in range(N_CORES)])
    k = np.stack([res.results[i]["k"] for i in range(N_CORES)])
    v = np.stack([res.results[i]["v"] for i in range(N_CORES)])
    return (out, k, v)
